# revision 21
# baseline (speedup 1.0000x reference)
"""Trainium2 kernel for nn_AttentionModel_PCA (embedding_lookup).

Math: with sf = softmax(Q^T K) per head,
  G[i,a,m] = sum_h sum_j sf[h,i,j] * V[h,a,Z2[j,m]]

Instead of the one-hot (j,c) blow-up (K=5376 dense GEMM, 118e9 MACs),
contract over (h,j) = 2048:
  G[i,(a,m)] = sum_{(h,j)} sfT[(h,j), i] * Vg[(h,j), (a,m)]
with Vg[(h,j),(a,m)] = V[h,a,Z2[j,m]] gathered on the host (cheap fancy
index from the tiny 21-entry V tables). That is 45e9 MACs total, 2.6x
fewer than the dense-E path, ~172k PE cycles/core in fp8 DoubleRow.

Device per core (M sharded, Ms=512): lhsT = sf tiles (512 KB, fully
resident -> only 16 LDWEIGHTS per a), rhs = Vg streamed in 21 a-slabs
of [128, 8kt, 2ko, 512m] fp8 (1.05 MB each) through an NS-slot ring,
overlapping DMA with the PE. Out G[(a,it), i128, m] in fp8e4. Host does
softmax/quantize/gather prep and the tail (take_along_axis, logsumexp,
loss, regularizer).

Scales: sf*16 and Vg*8 keep fp8e4m3 in range (max ~16 / ~30 << 240);
PSUM accumulates fp32; the PSUM->SBUF copy multiplies by 1/128.
"""

import sys

import numpy as np
import ml_dtypes

for _p in ("/opt/trn_rl_repo",):
    if _p not in sys.path:
        sys.path.append(_p)

H, d, N1, N2, q1, q2, M = 8, 64, 256, 256, 21, 21, 4096
NCORES = 8
MS = M // NCORES          # 512 samples per core
KT = 8                    # contraction tiles over (h,j): kt == h
KO = 2                    # DoubleRow pair dim (j high bit)
IT = 2                    # i tiles (256 / 128)
NA = q1                   # 21 a iterations
NU = NA * IT              # 42 units (a-major, itile inner)
NS = 6                    # Vg a-slab ring slots
NB = 8                    # PSUM banks ping-ponged
NOT = 6                   # output ring slots
LAMBD = 0.001

SF_SCALE = 16.0
VG_SCALE = 8.0
DEQ = 1.0 / (SF_SCALE * VG_SCALE)

_PROGRAMS = {}


def _build_program():
    """Raw bass pipeline: explicit standalone wait_ge + then_inc.

    sync  : sfW load, then 21 per-a Vg slab loads (NS-slot ring)
    tensor: 21 a x 2 itiles x 8 accumulating DR matmuls, NB PSUM banks
    vector: PSUM -> SBUF dequant copies (bf16, NOT-slot out ring)
    scalar: SBUF -> G stores (HWDGE)
    """
    import concourse.bass as bass
    import concourse.mybir as mybir

    nc = bass.Bass()
    f8 = mybir.dt.float8e4
    bf16 = mybir.dt.bfloat16
    f32 = mybir.dt.float32
    perf = mybir.MatmulPerfMode.DoubleRow

    # host-preswizzled layouts (partition dim first, fully contiguous):
    #   sfW[ki][kt][ko][it][is] : lhsT tile for (kt, it) = [128, 2, 128]
    #   Vg [a][ki][kt][ko][m]   : rhs slab for a = [128, 8*2*512]
    sfW = nc.declare_dram_parameter("sfW", [128, KT * KO * IT * 128], f8,
                                    isOutput=False)
    Vg = nc.declare_dram_parameter("Vg", [NA, 128, KT * KO * MS], f8,
                                   isOutput=False)
    G = nc.declare_dram_parameter("G", [NU, 128, MS], f8, isOutput=True)

    from contextlib import ExitStack

    with ExitStack() as stack:
        ent = stack.enter_context
        sf_sb = ent(nc.sbuf_tensor([128, KT, KO, IT, 128], f8))
        slab = ent(nc.sbuf_tensor([128, NS, KT, KO, MS], f8))
        ot = ent(nc.sbuf_tensor([128, NOT, MS], f8))
        acc = ent(nc.psum_tensor([128, NB * MS], f32))
        # per-slot DMA sems stay exact (16 SDMA engines increment them
        # independently). A wait is only exact for the TOTAL increment
        # count of every transfer ever queued on that sem up to the wait
        # point - a prefix of in-flight transfers can fake a count. So the
        # two first-load phases get disjoint sems: (sfWa, s0a) on slot 0's
        # sem, (sfWb, s0b) on ld2.
        sl_sems = [ent(nc.semaphore(f"sl{s}")) for s in range(NS)]
        ld2_sem = ent(nc.semaphore("ld2"))
        st_sems = [ent(nc.semaphore(f"st{s}")) for s in range(NOT)]
        pe_cnt = ent(nc.semaphore("pe_cnt"))   # units fully consumed by PE
        cp_sem = ent(nc.semaphore("cp_sem"))   # PSUM->SBUF copies done
        block = ent(nc.Block())

        KH = KT // 2                     # first-phase kt half
        HB = KH * KO * MS                # Vg bytes/partition per half

        @block.sync
        def _(sync):
            # One ring, priority order: (sfW, slab0) split in kt-halves so
            # the PE can start on kt0-3 while kt4-7 is still in flight.
            # Everything on the same queue so slab0 is never starved by
            # round-robin against later slabs.
            sync.dma_start(sf_sb[:, :KH], sfW[:, :KH * KO * IT * 128]
                           ).then_inc(sl_sems[0], 16)
            sync.dma_start(slab[:, 0, :KH], Vg[0, :, :HB]).then_inc(sl_sems[0], 16)
            sync.dma_start(sf_sb[:, KH:], sfW[:, KH * KO * IT * 128:]
                           ).then_inc(ld2_sem, 16)
            sync.dma_start(slab[:, 0, KH:], Vg[0, :, HB:]).then_inc(ld2_sem, 16)
            for a in range(1, NA):
                if a >= NS:
                    # slab slot reused from a-NS: both its units consumed
                    sync.wait_ge(pe_cnt, (a - NS + 1) * IT)
                sync.dma_start(slab[:, a % NS], Vg[a, :, :]
                               ).then_inc(sl_sems[a % NS], 16)

        @block.tensor
        def _(tensor):
            # ~5 us of garbage warmup matmuls while the first loads are in
            # flight: ramps the PE out of its low/mid p-states so the real
            # stream runs at 2.4 GHz from the start. Banks are overwritten
            # by the real start=True matmuls, so the values never escape.
            for w in range(16):
                nc.tensor.matmul(
                    acc[:, (w % NB) * MS:(w % NB) * MS + MS],
                    ot[:, 2:4, :128], ot[:, 0:2, :],
                    start=True, stop=True, perf_mode=perf,
                )
            # a = 0 in two kt-half phases, tracking the split first loads
            for half in range(2):
                tensor.wait_ge(ld2_sem if half else sl_sems[0], 32)
                for it in range(IT):
                    for kt in range(KH * half, KH * (half + 1)):
                        mm = nc.tensor.matmul(
                            acc[:, it * MS:it * MS + MS],
                            sf_sb[:, kt, :, it, :],
                            slab[:, 0, kt, :, :],
                            start=(kt == 0), stop=(kt == KT - 1),
                            perf_mode=perf,
                        )
                        if kt == KT - 1:
                            mm.then_inc(pe_cnt, 1)
            for a in range(1, NA):
                # slot 0's sem carries sfWa+s0a = 32 from the first loads
                tensor.wait_ge(sl_sems[a % NS],
                               16 * (a // NS + 1) + (16 if a % NS == 0 else 0))
                for it in range(IT):
                    u = a * IT + it
                    b = (u % NB) * MS
                    if u >= NB:
                        tensor.wait_ge(cp_sem, u - NB + 1)
                    for kt in range(KT):
                        mm = nc.tensor.matmul(
                            acc[:, b:b + MS],
                            sf_sb[:, kt, :, it, :],
                            slab[:, a % NS, kt, :, :],
                            start=(kt == 0), stop=(kt == KT - 1),
                            perf_mode=perf,
                        )
                        if kt == KT - 1:
                            mm.then_inc(pe_cnt, 1)

        @block.vector
        def _(vector):
            for u in range(NU):
                vector.wait_ge(pe_cnt, u + 1)
                if u >= NOT:
                    # ot slot reused from u-NOT: wait for its store
                    vector.wait_ge(st_sems[u % NOT], 16 * (u // NOT))
                nc.vector.tensor_scalar_mul(
                    ot[:, u % NOT, :], acc[:, (u % NB) * MS:(u % NB + 1) * MS],
                    DEQ,
                ).then_inc(cp_sem, 1)

        @block.scalar
        def _(scalar):
            for u in range(NU):
                scalar.wait_ge(cp_sem, u + 1)
                scalar.dma_start(
                    G[u, :, :], ot[:, u % NOT, :]
                ).then_inc(st_sems[u % NOT], 16)

    return nc


def host_prep(Q, K, V, Z2):
    """softmax, quantized+preswizzled sf weights and per-core Vg slabs."""
    e = np.einsum("hdi,hdj->hij", Q, K, optimize=True)
    e -= e.max(axis=2, keepdims=True)
    np.exp(e, out=e)
    sf = e / e.sum(axis=2, keepdims=True)

    # sfW[ki][kt=h][ko][it][is] = (sf*16)[h, it*128+is, ko*128+ki]
    sf8 = (sf * SF_SCALE).astype(ml_dtypes.float8_e4m3)
    sfW = np.ascontiguousarray(
        sf8.reshape(H, IT, 128, KO, 128).transpose(4, 0, 3, 1, 2)
    ).reshape(128, KT * KO * IT * 128)

    # Vg[a][ki][kt=h][ko][m] = (V*8)[h, a, Z2[ko*128+ki, m]]
    V8 = (V * VG_SCALE).astype(ml_dtypes.float8_e4m3)
    Vgf = V8[:, :, Z2.astype(np.int64)]            # (H, q1, N2, M)
    return sf, sf8, V8, sfW, Vgf


def build_vg(Vgf_c):
    """Per-core Vg slab tensor [NA, 128, KT*KO*MS] from (H, q1, N2, Ms)."""
    Mloc = Vgf_c.shape[3]
    return np.ascontiguousarray(
        Vgf_c.reshape(H, NA, KO, 128, Mloc).transpose(1, 3, 0, 2, 4)
    ).reshape(NA, 128, KT * KO * Mloc)


def host_tail(G, sf, V, Z1, weights):
    """take_along_axis + logsumexp + loss + regularizer on (N1, M, q1) G."""
    Z1i = Z1.astype(np.int64)
    mat_ene_sum = np.take_along_axis(G, Z1i[:, :, None], axis=2)[..., 0].sum(axis=0)

    Gm = G.max(axis=0)                                   # (M, q1)
    L = np.log(np.exp(G - Gm).sum(axis=0)) + Gm          # (M, q1)
    mx = np.maximum(L.max(axis=1), 0.0)
    logZ = np.log(np.exp(L - mx[:, None]).sum(axis=1)
                  + (N1 - q1) * np.exp(-mx)) + mx

    pl = -(weights.astype(np.float64)
           * (mat_ene_sum.astype(np.float64) - logZ.astype(np.float64))).sum()

    sf2 = sf.reshape(H, -1).astype(np.float64)
    VV = V.reshape(H, -1).astype(np.float64)
    reg = LAMBD * ((sf2 @ sf2.T) * (VV @ VV.T)).sum()
    return np.array(pl + reg, dtype=np.float32)


def run_device(sfW, Vgf, trace=False, **kw):
    from concourse.bass_utils import run_bass_kernel_spmd

    if "prog" not in _PROGRAMS:
        _PROGRAMS["prog"] = _build_program()
    in_maps = [
        {"sfW": sfW, "Vg": build_vg(Vgf[:, :, :, c * MS:(c + 1) * MS])}
        for c in range(NCORES)
    ]
    out = run_bass_kernel_spmd(_PROGRAMS["prog"], in_maps, list(range(NCORES)),
                               trace=trace, **kw)
    # G[u= a*2+it][i128][m] -> (N1, q1, Mloc) -> concat m
    Gf = np.concatenate(
        [np.asarray(out.results[c]["G"]).astype(np.float32)
         .reshape(NA, IT, 128, MS).transpose(1, 2, 0, 3).reshape(N1, NA, MS)
         for c in range(NCORES)],
        axis=2)                                          # (N1, q1, M)
    return Gf, out


def kernel(**inputs):
    Q = np.asarray(inputs["Q"], np.float32)
    K = np.asarray(inputs["K"], np.float32)
    V = np.asarray(inputs["V"], np.float32)
    Z1 = np.asarray(inputs["Z1"])
    Z2 = np.asarray(inputs["Z2"])
    weights = np.asarray(inputs["weights"], np.float32)

    sf, _, _, sfW, Vgf = host_prep(Q, K, V, Z2)
    Gf, _ = run_device(sfW, Vgf)
    G = Gf.transpose(0, 2, 1)                            # (N1, M, q1)
    return host_tail(G, sf, V, Z1, weights)


# revision 23
# speedup vs baseline: 1.0620x; 1.0620x over previous
"""Trainium2 kernel for nn_AttentionModel_PCA (embedding_lookup).

Math: with sf = softmax(Q^T K) per head,
  G[i,a,m] = sum_h sum_j sf[h,i,j] * V[h,a,Z2[j,m]]

Instead of the one-hot (j,c) blow-up (K=5376 dense GEMM, 118e9 MACs),
contract over (h,j) = 2048:
  G[i,(a,m)] = sum_{(h,j)} sfT[(h,j), i] * Vg[(h,j), (a,m)]
with Vg[(h,j),(a,m)] = V[h,a,Z2[j,m]] gathered on the host (cheap fancy
index from the tiny 21-entry V tables). That is 45e9 MACs total, 2.6x
fewer than the dense-E path, ~172k PE cycles/core in fp8 DoubleRow.

Device per core (M sharded, Ms=512): lhsT = sf tiles (512 KB, fully
resident -> only 16 LDWEIGHTS per a), rhs = Vg streamed in 21 a-slabs
of [128, 8kt, 2ko, 512m] fp8 (1.05 MB each) through an NS-slot ring,
overlapping DMA with the PE. Out G[(a,it), i128, m] in fp8e4. Host does
softmax/quantize/gather prep and the tail (take_along_axis, logsumexp,
loss, regularizer).

Scales: sf*16 and Vg*8 keep fp8e4m3 in range (max ~16 / ~30 << 240);
PSUM accumulates fp32; the PSUM->SBUF copy multiplies by 1/128.
"""

import sys

import numpy as np
import ml_dtypes

for _p in ("/opt/trn_rl_repo",):
    if _p not in sys.path:
        sys.path.append(_p)

H, d, N1, N2, q1, q2, M = 8, 64, 256, 256, 21, 21, 4096
NCORES = 8
MS = M // NCORES          # 512 samples per core
KT = 8                    # contraction tiles over (h,j): kt == h
KO = 2                    # DoubleRow pair dim (j high bit)
IT = 2                    # i tiles (256 / 128)
NA = q1                   # 21 a iterations
NU = NA * IT              # 42 units (a-major, itile inner)
NS = 8                    # Vg a-slab ring slots
NB = 8                    # PSUM banks ping-ponged
NOT = 6                   # output ring slots
LAMBD = 0.001

SF_SCALE = 16.0
VG_SCALE = 8.0
DEQ = 1.0 / (SF_SCALE * VG_SCALE)

_PROGRAMS = {}


def _build_program():
    """Raw bass pipeline: explicit standalone wait_ge + then_inc.

    sync  : sfW load, then 21 per-a Vg slab loads (NS-slot ring)
    tensor: 21 a x 2 itiles x 8 accumulating DR matmuls, NB PSUM banks
    vector: PSUM -> SBUF dequant copies (bf16, NOT-slot out ring)
    scalar: SBUF -> G stores (HWDGE)
    """
    import concourse.bass as bass
    import concourse.mybir as mybir

    nc = bass.Bass()
    f8 = mybir.dt.float8e4
    bf16 = mybir.dt.bfloat16
    f32 = mybir.dt.float32
    perf = mybir.MatmulPerfMode.DoubleRow

    # host-preswizzled layouts (partition dim first, fully contiguous):
    #   sfW[ki][kt][ko][it][is] : lhsT tile for (kt, it) = [128, 2, 128]
    #   Vg [a][ki][kt][ko][m]   : rhs slab for a = [128, 8*2*512]
    sfW = nc.declare_dram_parameter("sfW", [128, KT * KO * IT * 128], f8,
                                    isOutput=False)
    Vg = nc.declare_dram_parameter("Vg", [NA, 128, KT * KO * MS], f8,
                                   isOutput=False)
    G = nc.declare_dram_parameter("G", [NU, 128, MS], f8, isOutput=True)

    from contextlib import ExitStack

    with ExitStack() as stack:
        ent = stack.enter_context
        sf_sb = ent(nc.sbuf_tensor([128, KT, KO, IT, 128], f8))
        slab = ent(nc.sbuf_tensor([128, NS, KT, KO, MS], f8))
        ot = ent(nc.sbuf_tensor([128, NOT, MS], f8))
        acc = ent(nc.psum_tensor([128, NB * MS], f32))
        # per-slot DMA sems stay exact (16 SDMA engines increment them
        # independently). A wait is only exact for the TOTAL increment
        # count of every transfer ever queued on that sem up to the wait
        # point - a prefix of in-flight transfers can fake a count. So the
        # two first-load phases get disjoint sems: (sfWa, s0a) on slot 0's
        # sem, (sfWb, s0b) on ld2.
        sl_sems = [ent(nc.semaphore(f"sl{s}")) for s in range(NS)]
        ld2_sem = ent(nc.semaphore("ld2"))
        st_sems = [ent(nc.semaphore(f"st{s}")) for s in range(NOT)]
        pe_cnt = ent(nc.semaphore("pe_cnt"))   # units fully consumed by PE
        cp_sem = ent(nc.semaphore("cp_sem"))   # PSUM->SBUF copies done
        block = ent(nc.Block())

        KH = KT // 2                     # first-phase kt half
        HB = KH * KO * MS                # Vg bytes/partition per half

        @block.sync
        def _(sync):
            # One ring, priority order: (sfW, slab0) split in kt-halves so
            # the PE can start on kt0-3 while kt4-7 is still in flight.
            # Everything on the same queue so slab0 is never starved by
            # round-robin against later slabs.
            sync.dma_start(sf_sb[:, :KH], sfW[:, :KH * KO * IT * 128]
                           ).then_inc(sl_sems[0], 16)
            sync.dma_start(slab[:, 0, :KH], Vg[0, :, :HB]).then_inc(sl_sems[0], 16)
            sync.dma_start(sf_sb[:, KH:], sfW[:, KH * KO * IT * 128:]
                           ).then_inc(ld2_sem, 16)
            sync.dma_start(slab[:, 0, KH:], Vg[0, :, HB:]).then_inc(ld2_sem, 16)
            for a in range(1, NA):
                if a >= NS:
                    # slab slot reused from a-NS: both its units consumed
                    sync.wait_ge(pe_cnt, (a - NS + 1) * IT)
                sync.dma_start(slab[:, a % NS], Vg[a, :, :]
                               ).then_inc(sl_sems[a % NS], 16)

        @block.tensor
        def _(tensor):
            # a = 0 in two kt-half phases, tracking the split first loads
            for half in range(2):
                tensor.wait_ge(ld2_sem if half else sl_sems[0], 32)
                for it in range(IT):
                    for kt in range(KH * half, KH * (half + 1)):
                        mm = nc.tensor.matmul(
                            acc[:, it * MS:it * MS + MS],
                            sf_sb[:, kt, :, it, :],
                            slab[:, 0, kt, :, :],
                            start=(kt == 0), stop=(kt == KT - 1),
                            perf_mode=perf,
                        )
                        if kt == KT - 1:
                            mm.then_inc(pe_cnt, 1)
            for a in range(1, NA):
                # slot 0's sem carries sfWa+s0a = 32 from the first loads
                tensor.wait_ge(sl_sems[a % NS],
                               16 * (a // NS + 1) + (16 if a % NS == 0 else 0))
                for it in range(IT):
                    u = a * IT + it
                    b = (u % NB) * MS
                    if u >= NB:
                        tensor.wait_ge(cp_sem, u - NB + 1)
                    for kt in range(KT):
                        mm = nc.tensor.matmul(
                            acc[:, b:b + MS],
                            sf_sb[:, kt, :, it, :],
                            slab[:, a % NS, kt, :, :],
                            start=(kt == 0), stop=(kt == KT - 1),
                            perf_mode=perf,
                        )
                        if kt == KT - 1:
                            mm.then_inc(pe_cnt, 1)

        @block.vector
        def _(vector):
            for u in range(NU):
                vector.wait_ge(pe_cnt, u + 1)
                if u >= NOT:
                    # ot slot reused from u-NOT: wait for its store
                    vector.wait_ge(st_sems[u % NOT], 16 * (u // NOT))
                nc.vector.tensor_scalar_mul(
                    ot[:, u % NOT, :], acc[:, (u % NB) * MS:(u % NB + 1) * MS],
                    DEQ,
                ).then_inc(cp_sem, 1)

        @block.scalar
        def _(scalar):
            for u in range(NU):
                scalar.wait_ge(cp_sem, u + 1)
                scalar.dma_start(
                    G[u, :, :], ot[:, u % NOT, :]
                ).then_inc(st_sems[u % NOT], 16)

    return nc


def host_prep(Q, K, V, Z2):
    """softmax, quantized+preswizzled sf weights and per-core Vg slabs."""
    e = np.einsum("hdi,hdj->hij", Q, K, optimize=True)
    e -= e.max(axis=2, keepdims=True)
    np.exp(e, out=e)
    sf = e / e.sum(axis=2, keepdims=True)

    # sfW[ki][kt=h][ko][it][is] = (sf*16)[h, it*128+is, ko*128+ki]
    sf8 = (sf * SF_SCALE).astype(ml_dtypes.float8_e4m3)
    sfW = np.ascontiguousarray(
        sf8.reshape(H, IT, 128, KO, 128).transpose(4, 0, 3, 1, 2)
    ).reshape(128, KT * KO * IT * 128)

    # Vg[a][ki][kt=h][ko][m] = (V*8)[h, a, Z2[ko*128+ki, m]]
    V8 = (V * VG_SCALE).astype(ml_dtypes.float8_e4m3)
    Vgf = V8[:, :, Z2.astype(np.int64)]            # (H, q1, N2, M)
    return sf, sf8, V8, sfW, Vgf


def build_vg(Vgf_c):
    """Per-core Vg slab tensor [NA, 128, KT*KO*MS] from (H, q1, N2, Ms)."""
    Mloc = Vgf_c.shape[3]
    return np.ascontiguousarray(
        Vgf_c.reshape(H, NA, KO, 128, Mloc).transpose(1, 3, 0, 2, 4)
    ).reshape(NA, 128, KT * KO * Mloc)


def host_tail(G, sf, V, Z1, weights):
    """take_along_axis + logsumexp + loss + regularizer on (N1, M, q1) G."""
    Z1i = Z1.astype(np.int64)
    mat_ene_sum = np.take_along_axis(G, Z1i[:, :, None], axis=2)[..., 0].sum(axis=0)

    Gm = G.max(axis=0)                                   # (M, q1)
    L = np.log(np.exp(G - Gm).sum(axis=0)) + Gm          # (M, q1)
    mx = np.maximum(L.max(axis=1), 0.0)
    logZ = np.log(np.exp(L - mx[:, None]).sum(axis=1)
                  + (N1 - q1) * np.exp(-mx)) + mx

    pl = -(weights.astype(np.float64)
           * (mat_ene_sum.astype(np.float64) - logZ.astype(np.float64))).sum()

    sf2 = sf.reshape(H, -1).astype(np.float64)
    VV = V.reshape(H, -1).astype(np.float64)
    reg = LAMBD * ((sf2 @ sf2.T) * (VV @ VV.T)).sum()
    return np.array(pl + reg, dtype=np.float32)


def run_device(sfW, Vgf, trace=False, **kw):
    from concourse.bass_utils import run_bass_kernel_spmd

    if "prog" not in _PROGRAMS:
        _PROGRAMS["prog"] = _build_program()
    in_maps = [
        {"sfW": sfW, "Vg": build_vg(Vgf[:, :, :, c * MS:(c + 1) * MS])}
        for c in range(NCORES)
    ]
    out = run_bass_kernel_spmd(_PROGRAMS["prog"], in_maps, list(range(NCORES)),
                               trace=trace, **kw)
    # G[u= a*2+it][i128][m] -> (N1, q1, Mloc) -> concat m
    Gf = np.concatenate(
        [np.asarray(out.results[c]["G"]).astype(np.float32)
         .reshape(NA, IT, 128, MS).transpose(1, 2, 0, 3).reshape(N1, NA, MS)
         for c in range(NCORES)],
        axis=2)                                          # (N1, q1, M)
    return Gf, out


def kernel(**inputs):
    Q = np.asarray(inputs["Q"], np.float32)
    K = np.asarray(inputs["K"], np.float32)
    V = np.asarray(inputs["V"], np.float32)
    Z1 = np.asarray(inputs["Z1"])
    Z2 = np.asarray(inputs["Z2"])
    weights = np.asarray(inputs["weights"], np.float32)

    sf, _, _, sfW, Vgf = host_prep(Q, K, V, Z2)
    Gf, _ = run_device(sfW, Vgf)
    G = Gf.transpose(0, 2, 1)                            # (N1, M, q1)
    return host_tail(G, sf, V, Z1, weights)
